# revision 36
# baseline (speedup 1.0000x reference)
"""Baseline kernel (original) — reconstructed for A/B thermal comparison."""

import sys
from contextlib import ExitStack

sys.path.insert(0, "/opt/trn_rl_repo")

import numpy as np

import concourse.bacc as bacc
import concourse.tile as tile
from concourse import mybir
from concourse.bass_utils import run_bass_kernel_spmd

EMBED = 1024
HEADS = 16
HEAD_DIM = 64
N_BATCH = 4
L = 1024
N_CORES = 8
HG = HEADS // 2
S = HG * HEAD_DIM
KT = EMBED // 128
MT = S // 128
F32 = mybir.dt.float32
F32R = mybir.dt.float32r
BF16 = mybir.dt.bfloat16
F8 = mybir.dt.float8e4
MM_DTYPE = "bf16"
SCALE = 1.0 / 32.0
SCALE64 = 1.0 / 2048.0

_CACHED = {}


def _build(apply_mask: bool):
    MMD = F32R if MM_DTYPE == "f32r" else BF16
    nc = bacc.Bacc("TRN2", target_bir_lowering=False, debug=False,
                   num_devices=N_CORES)

    xqT = nc.dram_tensor("xqT", [EMBED, L], F8, kind="ExternalInput").ap()
    xkT = nc.dram_tensor("xkT", [EMBED, L], F8, kind="ExternalInput").ap()
    xvT = nc.dram_tensor("xvT", [EMBED, L], MMD, kind="ExternalInput").ap()
    wqT = nc.dram_tensor("wqT", [EMBED, S], F8, kind="ExternalInput").ap()
    wkT = nc.dram_tensor("wkT", [EMBED, S], F8, kind="ExternalInput").ap()
    wvT = nc.dram_tensor("wvT", [EMBED, S], MMD, kind="ExternalInput").ap()
    woT = nc.dram_tensor("woT", [S, EMBED], MMD, kind="ExternalInput").ap()
    bq_d = nc.dram_tensor("bq", [128, MT], F32, kind="ExternalInput").ap()
    bk_d = nc.dram_tensor("bk", [128, MT], F32, kind="ExternalInput").ap()
    bv_d = nc.dram_tensor("bv", [1, S], MMD, kind="ExternalInput").ap()
    ones_d = nc.dram_tensor("ones", [128, 128], MMD, kind="ExternalInput").ap()
    if apply_mask:
        mb_d = nc.dram_tensor("maskbT", [L, L], F32, kind="ExternalInput").ap()
    out_d = nc.dram_tensor("out_partial", [L, EMBED], MMD,
                           kind="ExternalOutput").ap()

    with tile.TileContext(nc) as tc, ExitStack() as ctx:
        sb = ctx.enter_context(tc.tile_pool(name="sb", bufs=2))
        ps = ctx.enter_context(tc.tile_pool(name="ps", bufs=2, space="PSUM"))
        dr = ctx.enter_context(tc.tile_pool(name="dr", bufs=2, space="DRAM"))
        p2 = ctx.enter_context(tc.tile_pool(name="p2", bufs=2))

        bq_sb = sb.tile([128, MT], F32, tag="bias")
        bk_sb = sb.tile([128, MT], F32, tag="bias")
        bv_sb = sb.tile([1, S], MMD, tag="bvrow")
        ones1 = sb.tile([1, 128], MMD, tag="ones1")
        nc.sync.dma_start(bq_sb[:], bq_d[:])
        nc.sync.dma_start(bk_sb[:], bk_d[:])
        nc.sync.dma_start(bv_sb[:], bv_d[:])
        nc.sync.dma_start(ones1[:], ones_d[0:1, :])

        p1_cm = tc.tile_pool(name="p1", bufs=2)
        p1 = p1_cm.__enter__()

        def load_chunk(src, width, tag, bufs, nm, c, dt=MMD, eng=None):
            t = p1.tile([128, 4 * width], dt, tag=tag, bufs=bufs,
                        name=f"{nm}{c}")
            (eng or nc.sync).dma_start(
                t[:].rearrange("p (k l) -> p k l", k=4),
                src[c * 512:(c + 1) * 512, :].rearrange(
                    "(k p) l -> p k l", p=128))
            return t, [t[:, k * width:(k + 1) * width] for k in range(4)]

        def load_wx(name, wsrc, xsrc, xnm, dt=MMD):
            wc, xc, wv, xv = [], [], [], []
            for c in range(2):
                t, views = load_chunk(wsrc, S, f"w_{name}", 2, f"w{name}", c,
                                      dt, nc.scalar)
                wc.append(t); wv += views
                t, views = load_chunk(xsrc, L, "x", 3, xnm, c, dt)
                xc.append(t); xv += views
            return wc, xc, wv, xv

        def proj_T_f8(x_chunks, w_chunks, bias_sb, out_tag):
            # fp8 DoubleRow: each matmul consumes a pair of 128-deep k-tiles
            outs = []
            for m in range(MT):
                p = ps.tile([128, L], F32, tag="pa", bufs=2)
                for ci in range(2):
                    cs = slice(ci * 512, (ci + 1) * 512)
                    for cki in range(2):
                        w3 = w_chunks[cki][:].rearrange(
                            "p (k l) -> p k l", k=4)
                        x3 = x_chunks[cki][:].rearrange(
                            "p (k l) -> p k l", k=4)
                        for pj in range(2):
                            nc.tensor.matmul(
                                p[:, cs],
                                w3[:, 2 * pj:2 * pj + 2,
                                   m * 128:(m + 1) * 128],
                                x3[:, 2 * pj:2 * pj + 2, cs],
                                start=(cki == 0 and pj == 0),
                                stop=(cki == 1 and pj == 1),
                                perf_mode=mybir.MatmulPerfMode.DoubleRow)
                o = sb.tile([128, L], MMD, tag=out_tag, bufs=MT)
                nc.vector.tensor_scalar_add(o[:], p[:], bias_sb[:, m:m + 1])
                outs.append(o)
            return outs

        warm_ps = ps.tile([128, 512], F32, tag="pb", bufs=4)
        for i in range(8):
            nc.tensor.matmul(warm_ps[:, 0:512], (ones1[:]),
                             (bv_sb[0:1, 0:512]), start=True, stop=True)

        wk_c, xk_c, _, _ = load_wx("k", wkT, xkT, "xk", F8)
        kT_t = proj_T_f8(xk_c, wk_c, bk_sb, "kT")
        wq_c, xq_c, _, _ = load_wx("q", wqT, xqT, "xq", F8)
        qT_t = proj_T_f8(xq_c, wq_c, bq_sb, "qT")

        wo_t = []
        for c in range(MT):
            t = p2.tile([128, EMBED], MMD, tag="wo", bufs=MT, name=f"wo{c}")
            nc.scalar.dma_start(t[:], woT[c * 128:(c + 1) * 128, :])
            wo_t.append(t)

        if apply_mask:
            mb_t = []
            for k in range(KT):
                t = p2.tile([128, L], F32, tag="mb", bufs=KT)
                nc.sync.dma_start(t[:], mb_d[k * 128:(k + 1) * 128, :])
                mb_t.append(t)

        xn_t = [sb.tile([128, L], MMD, tag="xn", bufs=MT, name=f"xn{i}")
                for i in range(MT)]

        def emit_qk_exp_j(m, j, pts):
                h = 2 * m + j
                rows = slice(j * 64, (j + 1) * 64)
                for k in range(KT):
                    e = ps.tile([128, L], F32, tag="pa", bufs=2,
                                name=f"e{h}_{k}")
                    for ch in range(2):
                        cs = slice(ch * 512, (ch + 1) * 512)
                        nc.tensor.matmul(
                            e[:, cs],
                            (kT_t[m][rows, k * 128:(k + 1) * 128]),
                            (qT_t[m][rows, cs]),
                            start=True, stop=True)
                    pt = p2.tile([128, L], MMD, tag="pt", bufs=32,
                                 name=f"pt{h}_{k}")
                    if apply_mask:
                        es = p2.tile([128, L], F32, tag="es", bufs=2,
                                     name=f"es{h}_{k}")
                        nc.vector.tensor_add(es[:], e[:], mb_t[k][:])
                        nc.scalar.activation(pt[:], es[:],
                                             mybir.ActivationFunctionType.Exp,
                                             scale=SCALE64)
                    else:
                        nc.scalar.activation(pt[:], e[:],
                                             mybir.ActivationFunctionType.Exp,
                                             scale=SCALE64)
                    pts[j].append(pt)

        def emit_avnorm_j(m, j, pts):
                h = 2 * m + j
                och = []
                for ch in range(2):
                    cs = slice(ch * 512, (ch + 1) * 512)
                    o = ps.tile([65, 512], F32, tag="pb", bufs=4,
                                name=f"o{h}_{ch}")
                    for k in range(KT):
                        nc.tensor.matmul(o[:],
                                         (v_t[k][:, h * 65:(h + 1) * 65]),
                                         (pts[j][k][:, cs]),
                                         start=(k == 0), stop=(k == KT - 1))
                    och.append(o)
                den = dr.tile([1, L], F32, tag="den", name=f"den{h}")
                if j == 1:
                    xtmp = p2.tile([64, L], MMD, tag="xtmp", bufs=2,
                                   name=f"xtmp{h}")
                for ch in range(2):
                    cs = slice(ch * 512, (ch + 1) * 512)
                    den_row = p2.tile([65, 512], F32, tag="rcprow", bufs=2,
                                      name=f"denrow{h}_{ch}")
                    nc.vector.tensor_copy(den_row[64:65, :],
                                          och[ch][64:65, :])
                    nc.sync.dma_start(den[0:1, cs], den_row[64:65, :])
                    den_b = p2.tile([64, 512], F32, tag="denb", bufs=2,
                                    name=f"denb{h}_{ch}")
                    nc.sync.dma_start(den_b[:],
                                      den[0:1, cs].to_broadcast((64, 512)))
                    rcp = p2.tile([64, 512], F32, tag="rcp", bufs=2,
                                  name=f"rcp{h}_{ch}")
                    nc.vector.reciprocal_approx_fast(rcp[:], den_b[:])
                    if j == 0:
                        nc.vector.tensor_mul(xn_t[m][0:64, cs],
                                             och[ch][0:64, :], rcp[:])
                    else:
                        nc.vector.tensor_mul(xtmp[:, cs], och[ch][0:64, :],
                                             rcp[:])
                        nc.sync.dma_start(
                            xn_t[m][64:128, cs], xtmp[:, cs])

        def emit_outproj():
            for qt in range(KT):
                qs = slice(qt * 128, (qt + 1) * 128)
                for ec in range(2):
                    es_ = slice(ec * 512, (ec + 1) * 512)
                    f = ps.tile([128, 512], F32, tag="pb", bufs=4,
                                name=f"f{qt}_{ec}")
                    for mi in range(MT):
                        nc.tensor.matmul(f[:], (xn_t[mi][:, qs]),
                                         (wo_t[mi][:, es_]),
                                         start=(mi == 0),
                                         stop=(mi == MT - 1))
                    os_ = sb.tile([128, 512], MMD, tag="osb", bufs=3,
                                  name=f"os{qt}_{ec}")
                    nc.vector.tensor_copy(os_[:], f[:])
                    eng = nc.sync if (qt + ec) % 2 == 0 else nc.gpsimd
                    eng.dma_start(out_d[qs, es_], os_[:])

        pts0 = {0: [], 1: []}
        pts1 = {0: [], 1: []}
        pts2 = {0: [], 1: []}
        pts3 = {0: [], 1: []}
        emit_qk_exp_j(0, 0, pts0)
        emit_qk_exp_j(0, 1, pts0)
        emit_qk_exp_j(1, 0, pts1)
        emit_qk_exp_j(1, 1, pts1)

        _, _, wv_t, xv_tiles = load_wx("v", wvT, xvT, "xv")
        v_t = []
        for mp in range(KT):
            p = ps.tile([128, S], F32, tag="pb", bufs=4)
            for k in range(KT):
                nc.tensor.matmul(p[:], (xv_tiles[k][:, mp * 128:(mp + 1) * 128]),
                                 (wv_t[k]), start=(k == 0), stop=False)
            nc.tensor.matmul(p[:], (ones1[:]), (bv_sb[:]),
                             start=False, stop=True)
            vb = sb.tile([128, HG * 65], MMD, tag="vb", bufs=KT)
            vb3 = vb[:].rearrange("p (h d) -> p h d", h=HG)
            nc.sync.dma_start(vb3[:, :, 64:65], ones_d[:, 0:HG].rearrange("p (h d) -> p h d", d=1))
            nc.vector.tensor_copy(vb3[:, :, 0:64],
                                  p[:].rearrange("p (h d) -> p h d", h=HG))
            v_t.append(vb)

        p1_cm.__exit__(None, None, None)

        emit_avnorm_j(0, 0, pts0)
        emit_avnorm_j(0, 1, pts0)
        emit_qk_exp_j(2, 0, pts2)
        emit_avnorm_j(1, 0, pts1)
        emit_qk_exp_j(2, 1, pts2)
        emit_avnorm_j(1, 1, pts1)
        emit_qk_exp_j(3, 0, pts3)
        emit_avnorm_j(2, 0, pts2)
        emit_qk_exp_j(3, 1, pts3)
        emit_avnorm_j(2, 1, pts2)
        emit_avnorm_j(3, 0, pts3)
        emit_avnorm_j(3, 1, pts3)
        emit_outproj()

    nc.compile()
    return nc


def make_in_maps(values, keys, queries, mask, Wv, bv, Wk, bk, Wq, bq, Wo, bo):
    values = np.asarray(values, dtype=np.float32)
    keys = np.asarray(keys, dtype=np.float32)
    queries = np.asarray(queries, dtype=np.float32)
    mask = np.asarray(mask)
    Wv, bv = np.asarray(Wv, np.float32), np.asarray(bv, np.float32)
    Wk, bk = np.asarray(Wk, np.float32), np.asarray(bk, np.float32)
    Wq = np.asarray(Wq, np.float32) * np.float32(8.0)
    bq = np.asarray(bq, np.float32) * np.float32(8.0)
    Wk8 = np.asarray(Wk, np.float32) * np.float32(8.0)
    bk8 = np.asarray(bk, np.float32) * np.float32(8.0)
    Wo = np.asarray(Wo, np.float32)

    apply_mask = not bool(np.all(mask != 0))
    import ml_dtypes
    mmd_np = ml_dtypes.bfloat16

    def ct(a):
        return np.ascontiguousarray(np.asarray(a, dtype=np.float32))

    def cm(a):
        return np.ascontiguousarray(np.asarray(a).astype(mmd_np))

    f8_np = ml_dtypes.float8_e4m3

    def c8(a):
        return np.ascontiguousarray(np.asarray(a).astype(f8_np))

    in_maps = []
    for c in range(N_CORES):
        n, g = c // 2, c % 2
        sl = slice(g * S, (g + 1) * S)
        m = {
            "xqT": c8(queries[n].T),
            "xkT": c8(keys[n].T),
            "xvT": cm(values[n].T),
            "wqT": c8(Wq[sl, :].T),
            "wkT": c8(Wk8[sl, :].T),
            "wvT": cm(Wv[sl, :].T),
            "woT": cm(Wo[:, sl].T),
            "bq": ct(bq[sl].reshape(MT, 128).T),
            "bk": ct(bk8[sl].reshape(MT, 128).T),
            "bv": cm(bv[sl].reshape(1, S)),
            "ones": np.ones((128, 128), mmd_np),
        }
        if apply_mask:
            mb = np.where(mask[n, 0] == 0, np.float32(-1e18), np.float32(0.0))
            m["maskbT"] = ct(mb.T)
        in_maps.append(m)
    return in_maps, apply_mask


def kernel(values, keys, queries, mask, Wv, bv, Wk, bk, Wq, bq, Wo, bo):
    in_maps, apply_mask = make_in_maps(values, keys, queries, mask, Wv, bv,
                                       Wk, bk, Wq, bq, Wo, bo)
    if apply_mask not in _CACHED:
        _CACHED[apply_mask] = _build(apply_mask)
    nc = _CACHED[apply_mask]

    res = run_bass_kernel_spmd(nc, in_maps, list(range(N_CORES))).results
    bo = np.asarray(bo, np.float32)
    out = np.empty((N_BATCH, L, EMBED), dtype=np.float32)
    for n in range(N_BATCH):
        out[n] = (res[2 * n]["out_partial"].astype(np.float32)
                  + res[2 * n + 1]["out_partial"].astype(np.float32)
                  + bo[None, :])
    return out
